# revision 1
# baseline (speedup 1.0000x reference)
"""Distributed Trainium2 Bass kernel for a dense transformer self-attention block.

Reference computation (fp32):
  normed = x * rsqrt(mean(x^2, -1) + 1e-5)
  qkv = normed @ w_qkv.T ; split into q,k (RoPE'd) and v, 16 heads x 128
  attn = softmax(causal(q k^T / sqrt(128)))
  out  = x + (attn @ v merged) @ w_out.T

Sharding across 8 NeuronCores: tensor-parallel by head (2 heads/core) for
QKV projection + attention; AllToAll converts the head-sharded attention
output to a sequence-sharded layout; the out-projection + residual run on
each core's 512-token slice.  Host-side gather is pure concatenation.
"""

import os
import sys

if "/opt/trn_rl_repo" not in sys.path:
    sys.path.insert(0, "/opt/trn_rl_repo")

import numpy as np
import ml_dtypes

import concourse.bass as bass
import concourse.mybir as mybir
from concourse import bacc, tile
from concourse.bass_utils import run_bass_kernel_spmd

EMB, KEY, HEAD, NH = 2048, 128, 128, 16
B, S = 2, 2048
NCORES = 8
HPC = NH // NCORES          # heads per core = 2
T = B * S                   # 4096 tokens
TPC = T // NCORES           # 512 tokens per core after A2A
TB = 512                    # token block for QKV phase
NTB = T // TB               # 8 blocks
QF = 512                    # query free-dim in attention
KT = 128                    # key tile (partition dim)
FCH = EMB // 128            # 16 feature chunks
MOUT = 6                    # qkv output tiles per core (q0 q1 k0 k1 v0 v1)
EPS = 1e-5
SOFTMAX_SCALE = 1.0 / float(np.sqrt(KEY))
NEG_BIG = -30000.0

BF16 = mybir.dt.bfloat16
F32 = mybir.dt.float32
F32R = mybir.dt.float32r
AX = mybir.AluOpType


def build_graph(mask_mode: str):
    """mask_mode: 'tril' (causal fast path), 'none' (no masking),
    'generic' (arbitrary additive mask input)."""
    nc = bacc.Bacc("TRN2", target_bir_lowering=False, debug=False,
                   num_devices=NCORES)

    # register the extra activation-bias constant (Ln bias = EPS)
    _ct = nc.alloc_sbuf_tensor(f"const-extra-eps", [128, 1], F32)
    nc.gpsimd.memset(_ct.ap(), EPS)
    nc.const_aps.aps[(F32, EPS)] = _ct.ap()
    nc.all_engine_barrier()

    xT = nc.dram_tensor("xT", [B, EMB, S], BF16, kind="ExternalInput")
    xres = nc.dram_tensor("xres", [TPC, EMB], F32, kind="ExternalInput")
    wqkvT = nc.dram_tensor("wqkvT", [EMB, MOUT * 128], BF16, kind="ExternalInput")
    woutT = nc.dram_tensor("woutT", [EMB, EMB], BF16, kind="ExternalInput")
    cosq = nc.dram_tensor("cosq", [KEY, S], BF16, kind="ExternalInput")
    sin2q = nc.dram_tensor("sin2q", [KEY, S], BF16, kind="ExternalInput")
    cosk = nc.dram_tensor("cosk", [KEY, S], BF16, kind="ExternalInput")
    sin2k = nc.dram_tensor("sin2k", [KEY, S], BF16, kind="ExternalInput")
    eye_d = nc.dram_tensor("eye", [128, 128], BF16, kind="ExternalInput")
    perm_d = nc.dram_tensor("perm", [128, 128], BF16, kind="ExternalInput")
    onescol_d = nc.dram_tensor("onescol", [128, 1], BF16, kind="ExternalInput")
    onesrow_d = nc.dram_tensor("onesrow", [1, 128], F32R, kind="ExternalInput")
    if mask_mode == "tril":
        dmask_d = nc.dram_tensor("dmask", [4, 128, QF], BF16, kind="ExternalInput")
    elif mask_mode == "generic":
        gmask_d = nc.dram_tensor("gmask", [S // KT, KT, S], BF16, kind="ExternalInput")
    out_ext = nc.dram_tensor("out", [TPC, EMB], F32, kind="ExternalOutput")

    # A2A split by head so the first collective overlaps the second head's
    # attention and the out-projection can start on half the features.
    a2a_in = [nc.dram_tensor(f"a2a_in{h}", [NCORES, HEAD, TPC], BF16)
              for h in range(HPC)]
    a2a_out = [nc.dram_tensor(f"a2a_out{h}", [NCORES, HEAD, TPC], BF16)
               for h in range(HPC)]

    # ---- static SBUF residents ----
    cos_sb = {}
    for name, dt_ in (("cosq", cosq), ("sin2q", sin2q), ("cosk", cosk), ("sin2k", sin2k)):
        cos_sb[name] = nc.alloc_sbuf_tensor(f"sb_{name}", [KEY, S], BF16).ap()
    eye_sb = nc.alloc_sbuf_tensor("sb_eye", [128, 128], BF16).ap()
    perm_sb = nc.alloc_sbuf_tensor("sb_perm", [128, 128], BF16).ap()
    onescol_sb = nc.alloc_sbuf_tensor("sb_onescol", [128, 1], BF16).ap()
    onesrow_sb = nc.alloc_sbuf_tensor("sb_onesrow", [1, 128], F32R).ap()
    if mask_mode == "tril":
        dmask_sb = nc.alloc_sbuf_tensor("sb_dmask", [128, 4, QF], BF16).ap()
    wqkv_sb = nc.alloc_sbuf_tensor("sb_wqkv", [128, FCH, MOUT * 128], BF16).ap()
    # q/k slabs: rope'd, [d, tok] per (head, block); v slabs token-major
    q_sb = {}
    k_sb = {}
    v_sb = {}
    for h in range(HPC):
        for tb in range(NTB):
            q_sb[(h, tb)] = nc.alloc_sbuf_tensor(f"sb_q_{h}_{tb}", [128, TB], BF16).ap()
            k_sb[(h, tb)] = nc.alloc_sbuf_tensor(f"sb_k_{h}_{tb}", [128, TB], BF16).ap()
            v_sb[(h, tb)] = nc.alloc_sbuf_tensor(f"sb_v_{h}_{tb}", [128, TB], BF16).ap()

    with tile.TileContext(nc) as tc:
        # constant loads
        for name, dt_ in (("cosq", cosq), ("sin2q", sin2q), ("cosk", cosk), ("sin2k", sin2k)):
            nc.sync.dma_start(cos_sb[name], dt_.ap())
        nc.sync.dma_start(eye_sb, eye_d.ap())
        nc.sync.dma_start(perm_sb, perm_d.ap())
        nc.sync.dma_start(onescol_sb, onescol_d.ap())
        nc.sync.dma_start(onesrow_sb, onesrow_d.ap())
        if mask_mode == "tril":
            nc.sync.dma_start(dmask_sb, dmask_d.ap().rearrange("t p f -> p t f"))
        for c in range(FCH):
            nc.sync.dma_start(wqkv_sb[:, c, :],
                              wqkvT.ap()[c * 128:(c + 1) * 128, :])

        from contextlib import ExitStack
        with ExitStack() as _stack:
            def _pool(name, bufs, **kw):
                return _stack.enter_context(tc.tile_pool(name=name, bufs=bufs, **kw))
            pool_x = _pool("xb", 17)
            pool_x2 = _pool("x2", 2)
            pool_small = _pool("small", 1)
            pool_nbc = _pool("nbc", 2)
            pool_rn = _pool("rn", 1)
            pool_rope = _pool("rope", 2)
            pool_pt = _pool("pt", 3)
            pool_oT = _pool("oT", 2)
            pool_sp = _pool("sp", 2)
            pool_gm = _pool("gm", 4)
            pool_aT = _pool("aT", 1)
            pool_we = _pool("we", 1)
            pool_res = _pool("res", 2)
            ps_sc = _pool("ps_sc", 2, space="PSUM")
            ps_pv = _pool("ps_pv", 2, space="PSUM")
            ps_sm = _pool("ps_sm", 1, space="PSUM")
            ps_misc = _pool("ps_misc", 3, space="PSUM")

            # preload the combined ln+exp activation table set once, so the
            # Ln/Exp mix never thrashes ACT_TABLE_LOADs
            from concourse.hw_specs import get_activation_tables
            _tables = list(get_activation_tables(nc.m.arch))
            _lnexp_id = _tables.index("natural_log_exp_and_others")
            nc.scalar.add_instruction(mybir.InstLoadActFuncSet(
                name="preload_lnexp", act_func_set_id=_lnexp_id, ins=[], outs=[]))

            # ================= Phase 1: norm + QKV + RoPE per token block ====
            def emit_block(tb):
                b = tb // 4
                s0 = (tb % 4) * TB
                xbs = []
                for c in range(FCH):
                    xc = pool_x.tile([128, TB], BF16, tag="xb", name=f"xb{tb}_{c}")
                    nc.sync.dma_start(xc[:], xT.ap()[b, c * 128:(c + 1) * 128,
                                                     s0:s0 + TB])
                    xbs.append(xc)

                # squares on DVE early; the ones-reduction matmuls are emitted
                # after QKV group m=0 so PE never waits on DVE
                x2s = []
                for c in range(FCH):
                    x2 = pool_x2.tile([128, TB], BF16, tag="x2", name=f"x2_{tb}_{c}")
                    nc.vector.tensor_tensor(x2[:], xbs[c][:], xbs[c][:], AX.mult)
                    x2s.append(x2)

                # QKV accumulation groups, with the norm chain + evictions
                # staggered one group behind so PE streams without stalls
                accs = {}

                def emit_evict(m):
                    acc = accs.pop(m)
                    if m < 4:
                        h = m % 2
                        is_q = m < 2
                        ctab = ropen["cosq" if is_q else "cosk"]
                        stab = ropen["sin2q" if is_q else "sin2k"]
                        a_t = pool_rope.tile([128, TB], BF16, tag="ropea",
                                             name=f"ra{tb}_{m}")
                        b_t = pool_rope.tile([128, TB], BF16, tag="ropeb",
                                             name=f"rb{tb}_{m}")
                        nc.vector.tensor_tensor(a_t[:], acc[:], ctab[:], AX.mult)
                        nc.vector.tensor_tensor(b_t[:], acc[:], stab[:], AX.mult)
                        rp = ps_sc.tile([128, TB], F32, tag="sc", name=f"rp{tb}_{m}")
                        nc.tensor.matmul(rp[:], perm_sb, b_t[:], start=True, stop=False)
                        nc.tensor.matmul(rp[:], eye_sb, a_t[:], start=False, stop=True)
                        dst = q_sb[(h, tb)] if is_q else k_sb[(h, tb)]
                        nc.vector.tensor_copy(dst[:], rp[:])
                    else:
                        h = m - 4
                        vtmp = pool_rope.tile([128, TB], BF16, tag="vtmp",
                                              name=f"vt{tb}_{m}")
                        nc.vector.tensor_tensor(vtmp[:], acc[:], nbc[:], AX.mult)
                        for j in range(TB // 128):
                            tp = ps_sc.tile([128, 128], BF16, tag="sc",
                                            name=f"tp{tb}_{m}_{j}")
                            nc.tensor.transpose(tp[:], vtmp[:, j * 128:(j + 1) * 128],
                                                eye_sb)
                            nc.vector.tensor_copy(
                                v_sb[(h, tb)][:, j * 128:(j + 1) * 128], tp[:])

                for m in range(MOUT):
                    acc = ps_misc.tile([128, TB], F32, tag="misc", name=f"acc{tb}_{m}")
                    for c in range(FCH):
                        nc.tensor.matmul(acc[:], wqkv_sb[:, c, m * 128:(m + 1) * 128],
                                         xbs[c][:], start=(c == 0), stop=(c == FCH - 1))
                    accs[m] = acc
                    if m == 0:
                        # sum-of-squares reduction (x2 ready by now)
                        ssq = ps_sm.tile([1, TB], F32, tag="sm", name=f"ssq{tb}")
                        for c in range(FCH):
                            nc.tensor.matmul(ssq[:], onescol_sb, x2s[c][:],
                                             start=(c == 0), stop=(c == FCH - 1))
                        lnt = pool_small.tile([1, TB], F32, tag="lnt", name=f"lnt{tb}")
                        nc.scalar.activation(lnt[:], ssq[:],
                                             mybir.ActivationFunctionType.Ln,
                                             bias=EPS, scale=1.0 / EMB)
                        invn = pool_small.tile([1, TB], F32R, tag="invn",
                                               name=f"invn{tb}")
                        with nc.allow_low_precision(reason="f32r bcast matmul"):
                            nc.scalar.activation(invn[:], lnt[:],
                                                 mybir.ActivationFunctionType.Exp,
                                                 scale=-0.5)
                    elif m == 1:
                        # norm broadcast + rope tables (ACT chain done during m=1)
                        nbc_ps = ps_sc.tile([128, TB], F32, tag="sc", name=f"nb{tb}")
                        nc.tensor.matmul(nbc_ps[:], onesrow_sb, invn[:],
                                         start=True, stop=True)
                        nbc = pool_nbc.tile([128, TB], BF16, tag="nbc", name=f"nbc{tb}")
                        nc.vector.tensor_copy(nbc[:], nbc_ps[:])
                        ropen = {}
                        for nm in ("cosq", "sin2q", "cosk", "sin2k"):
                            rt = pool_rn.tile([128, TB], BF16, tag=f"rn_{nm}",
                                               name=f"rn{tb}_{nm}")
                            nc.vector.tensor_tensor(rt[:], cos_sb[nm][:, s0:s0 + TB],
                                                    nbc[:], AX.mult)
                            ropen[nm] = rt
                    else:
                        emit_evict(m - 2)
                emit_evict(MOUT - 2)
                emit_evict(MOUT - 1)

            for tb in range(NTB):
                emit_block(tb)

            # ================= Phase 2: attention, h outer ===================
            def emit_attention(b, h, qb):
                tb_q = b * 4 + qb
                nkt = (4 * qb + 4) if mask_mode == "tril" else (S // KT)
                pacc = ps_pv.tile([128, QF], F32, tag="pacc", name=f"pv{b}_{h}_{qb}")
                dacc = ps_sm.tile([1, QF], F32, tag="sm", name=f"da{b}_{h}_{qb}")
                pts = {}

                def emit_scores(kt):
                    sc = ps_sc.tile([128, QF], F32, tag="sc", name=f"sc{b}{h}{qb}_{kt}")
                    is_diag = mask_mode == "tril" and kt >= 4 * qb
                    need_mask = is_diag or mask_mode == "generic"
                    nc.tensor.matmul(sc[:], k_sb[(h, b * 4 + kt // 4)][:, (kt % 4) * 128:(kt % 4) * 128 + 128],
                                     q_sb[(h, tb_q)][:],
                                     start=True, stop=not need_mask)
                    if is_diag:
                        nc.tensor.matmul(sc[:], eye_sb, dmask_sb[:, kt - 4 * qb, :],
                                         start=False, stop=True)
                    elif mask_mode == "generic":
                        gm = pool_gm.tile([128, QF], BF16, tag="gm",
                                          name=f"gm{b}{h}{qb}_{kt}")
                        nc.sync.dma_start(
                            gm[:], gmask_d.ap()[kt, :, qb * QF:(qb + 1) * QF])
                        nc.tensor.matmul(sc[:], eye_sb, gm[:], start=False, stop=True)
                    pt = pool_pt.tile([128, QF], BF16, tag="pt",
                                      name=f"pt{b}{h}{qb}_{kt}")
                    nc.scalar.activation(pt[:], sc[:],
                                         mybir.ActivationFunctionType.Exp,
                                         scale=SOFTMAX_SCALE)
                    pts[kt] = pt

                def emit_pv(kt):
                    pt = pts.pop(kt)
                    tb_k = b * 4 + kt // 4
                    kc = (kt % 4) * 128
                    nc.tensor.matmul(pacc[:], v_sb[(h, tb_k)][:, kc:kc + 128],
                                     pt[:], start=(kt == 0), stop=(kt == nkt - 1))
                    nc.tensor.matmul(dacc[:], onescol_sb, pt[:],
                                     start=(kt == 0), stop=(kt == nkt - 1))

                # scores run 2 tiles ahead of the PV/denominator consumers
                LAG = int(os.environ.get('K_LAG', '2'))
                for kt in range(nkt):
                    emit_scores(kt)
                    if kt >= LAG:
                        emit_pv(kt - LAG)
                for kt in range(max(0, nkt - LAG), nkt):
                    emit_pv(kt)

                # 1/denominator via exp(-ln(x)) on ACT (DVE reciprocal is slow)
                lnd = pool_small.tile([1, QF], F32, tag="lnd", name=f"ld{b}{h}{qb}")
                nc.scalar.activation(lnd[:], dacc[:], mybir.ActivationFunctionType.Ln)
                rec = pool_small.tile([1, QF], F32R, tag="rec", name=f"rc{b}{h}{qb}")
                with nc.allow_low_precision(reason="f32r rounding for bcast matmul"):
                    nc.scalar.activation(rec[:], lnd[:],
                                         mybir.ActivationFunctionType.Exp, scale=-1.0)
                rb_ps = ps_misc.tile([128, QF], F32, tag="misc", name=f"rp{b}{h}{qb}")
                nc.tensor.matmul(rb_ps[:], onesrow_sb, rec[:], start=True, stop=True)
                rb = pool_nbc.tile([128, QF], BF16, tag="rb", name=f"rb{b}{h}{qb}")
                nc.vector.tensor_copy(rb[:], rb_ps[:])
                oT = pool_oT.tile([128, QF], BF16, tag="oT", name=f"oT{b}{h}{qb}")
                nc.vector.tensor_tensor(oT[:], pacc[:], rb[:], AX.mult)
                nc.sync.dma_start(a2a_in[h].ap()[4 * b + qb, :, :], oT[:])

            aT = [None] * FCH

            def emit_a2a(h):
                nc.gpsimd.collective_compute(
                    "AllToAll", AX.bypass,
                    replica_groups=[list(range(NCORES))],
                    ins=[a2a_in[h][:]], outs=[a2a_out[h][:]],
                )
                for r in range(NCORES):
                    c = 2 * r + h
                    t = pool_aT.tile([128, TPC], BF16, tag=f"aT{c}", name=f"aT{c}")
                    nc.sync.dma_start(t[:], a2a_out[h].ap()[r, :, :])
                    aT[c] = t

            wtiles = {}

            def emit_wout_loads():
                for e in range(EMB // 512):
                    for c in range(FCH):
                        wt = pool_we.tile([128, 512], BF16, tag=f"we{c}_{e % 2}",
                                          name=f"we{e}_{c}")
                        nc.sync.dma_start(
                            wt[:], woutT.ap()[c * 128:(c + 1) * 128,
                                              e * 512:(e + 1) * 512])
                        wtiles[(e, c)] = wt

            for h in range(HPC):
                for b in range(B):
                    for qb in range(S // QF):
                        emit_attention(b, h, qb)
                emit_a2a(h)
                if h == 0:
                    emit_wout_loads()

            # ================= Phase 3: out-projection =======================
            # head-0 feature half accumulates right after attention (A2A #1 is
            # long done) and spills to SBUF; the head-1 half runs next, hiding
            # A2A #2 latency entirely, then adds the spill + residual.
            for e in range(EMB // 512):
                if e % 2 == 0:
                    continue
                # pass 1: even feature chunks (head 0 of each rank) -> spill
                spills = {}
                for ep in (e - 1, e):
                    for tt in range(TPC // 128):
                        acc = ps_sc.tile([128, 512], F32, tag="sc",
                                         name=f"oe{ep}_{tt}")
                        evens = [c for c in range(FCH) if c % 2 == 0]
                        for i, c in enumerate(evens):
                            nc.tensor.matmul(acc[:], aT[c][:, tt * 128:(tt + 1) * 128],
                                             wtiles[(ep, c)][:], start=(i == 0),
                                             stop=(i == len(evens) - 1))
                        sp = pool_sp.tile([128, 512], BF16, tag=f"sp{tt}",
                                          name=f"sp{ep}_{tt}")
                        nc.vector.tensor_copy(sp[:], acc[:])
                        spills[(ep, tt)] = sp
                # pass 2: odd feature chunks + spill + residual
                for ep in (e - 1, e):
                    for tt in range(TPC // 128):
                        acc = ps_pv.tile([128, 512], F32, tag="pacc",
                                         name=f"oo{ep}_{tt}")
                        odds = [c for c in range(FCH) if c % 2 == 1]
                        for i, c in enumerate(odds):
                            nc.tensor.matmul(acc[:], aT[c][:, tt * 128:(tt + 1) * 128],
                                             wtiles[(ep, c)][:], start=(i == 0),
                                             stop=(i == len(odds) - 1))
                        res_t = pool_res.tile([128, 512], F32, tag="res",
                                              name=f"rs{ep}_{tt}")
                        nc.sync.dma_start(
                            res_t[:], xres.ap()[tt * 128:(tt + 1) * 128,
                                                ep * 512:(ep + 1) * 512])
                        o1 = pool_res.tile([128, 512], F32, tag="o1", bufs=1,
                                           name=f"o1{ep}_{tt}")
                        nc.vector.tensor_tensor(o1[:], acc[:],
                                                spills.pop((ep, tt))[:], AX.add)
                        o = pool_res.tile([128, 512], F32, tag="o", name=f"o{ep}_{tt}")
                        nc.vector.tensor_tensor(o[:], o1[:], res_t[:], AX.add)
                        nc.sync.dma_start(
                            out_ext.ap()[tt * 128:(tt + 1) * 128,
                                         ep * 512:(ep + 1) * 512], o[:])

    nc.compile()
    return nc


_GRAPH_CACHE = {}


def _get_graph(mask_mode):
    if mask_mode not in _GRAPH_CACHE:
        _GRAPH_CACHE[mask_mode] = build_graph(mask_mode)
    return _GRAPH_CACHE[mask_mode]


def kernel(**inputs):
    emb = np.asarray(inputs["embeddings"], dtype=np.float32)       # [B, S, EMB]
    cos = np.asarray(inputs["cos_buffer"], dtype=np.float32)       # [2,1,1,S,KEY]
    sin = np.asarray(inputs["sin_buffer"], dtype=np.float32)
    causal = np.asarray(inputs["causal_buffer"])[0, 0]             # [S, S] bool
    wqkv = np.asarray(inputs["w_qkv"], dtype=np.float32)           # [6144, EMB]
    wout = np.asarray(inputs["w_out"], dtype=np.float32)           # [EMB, EMB]

    tril = np.tril(np.ones((S, S), dtype=bool))
    if np.array_equal(causal, tril):
        mask_mode = "tril"
    elif causal.all():
        mask_mode = "none"
    else:
        mask_mode = "generic"

    nc = _get_graph(mask_mode)

    bf = ml_dtypes.bfloat16
    xT = np.ascontiguousarray(emb.transpose(0, 2, 1)).astype(bf)   # [B, EMB, S]
    x2d = emb.reshape(T, EMB)
    wq = wqkv[0:EMB].reshape(NH, KEY, EMB)
    wk = wqkv[EMB:2 * EMB].reshape(NH, KEY, EMB)
    wv = wqkv[2 * EMB:].reshape(NH, HEAD, EMB)
    woutT_a = np.ascontiguousarray(wout.T).astype(bf)
    cosq_a = np.ascontiguousarray(cos[0, 0, 0].T).astype(bf)
    sin2q_a = np.ascontiguousarray(np.roll(sin[0, 0, 0].T, -64, axis=0)).astype(bf)
    cosk_a = np.ascontiguousarray(cos[1, 0, 0].T).astype(bf)
    sin2k_a = np.ascontiguousarray(np.roll(sin[1, 0, 0].T, -64, axis=0)).astype(bf)
    eye_a = np.eye(128, dtype=np.float32).astype(bf)
    perm_a = np.roll(np.eye(128, dtype=np.float32), 64, axis=0).astype(bf)
    onescol_a = np.ones((128, 1), np.float32).astype(bf)
    onesrow_a = np.ones((1, 128), np.float32)

    if mask_mode == "tril":
        p = np.arange(128)[:, None]
        j = np.arange(QF)[None, :]
        dmask_a = np.stack(
            [np.where(j < p + 128 * t, NEG_BIG, 0.0) for t in range(4)]
        ).astype(bf)                                               # [4, 128, QF]
    elif mask_mode == "generic":
        # additive mask in [kt, p, q] layout: keep where causal[q, k]
        cz = causal.T.reshape(S // KT, KT, S)                      # [kt, p(k), q]
        gmask_a = np.where(cz, 0.0, NEG_BIG).astype(bf)

    in_maps = []
    for c in range(NCORES):
        h0, h1 = 2 * c, 2 * c + 1
        wshard = np.concatenate(
            [wq[h0], wq[h1], wk[h0], wk[h1], wv[h0], wv[h1]], axis=0)   # [768, EMB]
        m = {
            "xT": xT,
            "xres": np.ascontiguousarray(x2d[c * TPC:(c + 1) * TPC]),
            "wqkvT": np.ascontiguousarray(wshard.T).astype(bf),
            "woutT": woutT_a,
            "cosq": cosq_a, "sin2q": sin2q_a, "cosk": cosk_a, "sin2k": sin2k_a,
            "eye": eye_a, "perm": perm_a,
            "onescol": onescol_a, "onesrow": onesrow_a,
        }
        if mask_mode == "tril":
            m["dmask"] = dmask_a
        elif mask_mode == "generic":
            m["gmask"] = gmask_a
        in_maps.append(m)

    trace = os.environ.get("BASS_KERNEL_PROFILE") == "1"
    res = run_bass_kernel_spmd(nc, in_maps, core_ids=list(range(NCORES)),
                               trace=trace)
    if trace:
        kernel.last_exec_time_ns = res.exec_time_ns
        kernel.last_results = res

    outs = [np.asarray(res.results[c]["out"], dtype=np.float32)
            for c in range(NCORES)]
    full = np.concatenate(outs, axis=0).reshape(B, S, EMB)
    return full



# revision 36
# speedup vs baseline: 1.0111x; 1.0111x over previous
"""Distributed Trainium2 Bass kernel for a dense transformer self-attention block.

Reference computation (fp32):
  normed = x * rsqrt(mean(x^2, -1) + 1e-5)
  qkv = normed @ w_qkv.T ; split into q,k (RoPE'd) and v, 16 heads x 128
  attn = softmax(causal(q k^T / sqrt(128)))
  out  = x + (attn @ v merged) @ w_out.T

Sharding across 8 NeuronCores: tensor-parallel by head (2 heads/core) for
QKV projection + attention; AllToAll converts the head-sharded attention
output to a sequence-sharded layout; the out-projection + residual run on
each core's 512-token slice.  Host-side gather is pure concatenation.

v2 notes (PE is the bottleneck: keep matmul count + columns minimal):
  - sum-of-squares via DVE accumulation + 1 matmul/block (was 16)
  - RoPE: one perm-matmul + DVE add (was perm+eye matmuls)
  - causal diag mask: multiplicative 0/1 on DVE post-exp (was PE matmul)
  - softmax denominator: DVE group-of-4 tree + 1 matmul/group (was 1/kt)
  - deferred per-group softmax finalize so PE never waits the ACT chain
  - w_out loads stream during phase 1; aT loads ride the gpsimd queue so
    the sync queue never head-of-line blocks oT stores
  - out-projection: all even-feature accumulations first (covers A2A#2)
"""

import os
import sys

if "/opt/trn_rl_repo" not in sys.path:
    sys.path.insert(0, "/opt/trn_rl_repo")

import numpy as np
import ml_dtypes

import concourse.bass as bass
import concourse.mybir as mybir
from concourse import bacc, tile
from concourse.bass_utils import run_bass_kernel_spmd

EMB, KEY, HEAD, NH = 2048, 128, 128, 16
B, S = 2, 2048
NCORES = 8
HPC = NH // NCORES          # heads per core = 2
T = B * S                   # 4096 tokens
TPC = T // NCORES           # 512 tokens per core after A2A
TB = 512                    # token block for QKV phase
NTB = T // TB               # 8 blocks
QF = 512                    # query free-dim in attention
KT = 128                    # key tile (partition dim)
FCH = EMB // 128            # 16 feature chunks
MOUT = 6                    # qkv output tiles per core (q0 q1 k0 k1 v0 v1)
EPS = 1e-5
SOFTMAX_SCALE = 1.0 / float(np.sqrt(KEY))
NEG_BIG = -30000.0

BF16 = mybir.dt.bfloat16
F32 = mybir.dt.float32
F32R = mybir.dt.float32r
AX = mybir.AluOpType


def build_graph(mask_mode: str, shared_rope: bool = True):
    """mask_mode: 'tril' (causal fast path), 'none' (no masking),
    'generic' (arbitrary additive mask input).  shared_rope: the q and k
    cos/sin tables are identical (true for the reference setup), so keep
    only one SBUF copy of each."""
    nc = bacc.Bacc("TRN2", target_bir_lowering=False, debug=False,
                   num_devices=NCORES)

    # register the extra activation-bias constant (Ln bias = EPS)
    _ct = nc.alloc_sbuf_tensor(f"const-extra-eps", [128, 1], F32)
    nc.gpsimd.memset(_ct.ap(), EPS)
    nc.const_aps.aps[(F32, EPS)] = _ct.ap()
    nc.all_engine_barrier()

    xT = nc.dram_tensor("xT", [B, EMB, S], BF16, kind="ExternalInput")
    xres = nc.dram_tensor("xres", [TPC, EMB], BF16, kind="ExternalInput")
    wqkvT = nc.dram_tensor("wqkvT", [EMB, MOUT * 128], BF16, kind="ExternalInput")
    woutT = nc.dram_tensor("woutT", [EMB, EMB], BF16, kind="ExternalInput")
    cos_tabs = [("cosq", nc.dram_tensor("cosq", [KEY, S], BF16,
                                        kind="ExternalInput")),
                ("sin2q", nc.dram_tensor("sin2q", [KEY, S], BF16,
                                         kind="ExternalInput"))]
    if not shared_rope:
        cos_tabs += [("cosk", nc.dram_tensor("cosk", [KEY, S], BF16,
                                             kind="ExternalInput")),
                     ("sin2k", nc.dram_tensor("sin2k", [KEY, S], BF16,
                                              kind="ExternalInput"))]
    eye_d = nc.dram_tensor("eye", [128, 128], BF16, kind="ExternalInput")
    perm_d = nc.dram_tensor("perm", [128, 128], BF16, kind="ExternalInput")
    onescol_d = nc.dram_tensor("onescol", [128, 1], BF16, kind="ExternalInput")
    onesrow_d = nc.dram_tensor("onesrow", [1, 128], F32R, kind="ExternalInput")
    if mask_mode == "tril":
        # single shifted mask base: window [384-128t : 896-128t] is the 0/1
        # multiplicative mask for diagonal tile t
        dmask_d = nc.dram_tensor("dmask", [128, 896], BF16, kind="ExternalInput")
    elif mask_mode == "generic":
        gmask_d = nc.dram_tensor("gmask", [S // KT, KT, S], BF16, kind="ExternalInput")
    out_ext = nc.dram_tensor("out", [TPC, EMB], F32, kind="ExternalOutput")

    # A2A split by head so the first collective overlaps the second head's
    # attention and the out-projection can start on half the features.
    a2a_in = [nc.dram_tensor(f"a2a_in{h}", [NCORES, HEAD, TPC], BF16)
              for h in range(HPC)]
    a2a_out = [nc.dram_tensor(f"a2a_out{h}", [NCORES, HEAD, TPC], BF16)
               for h in range(HPC)]

    # ---- static SBUF residents ----
    cos_sb = {}
    for name, _ in cos_tabs:
        cos_sb[name] = nc.alloc_sbuf_tensor(f"sb_{name}", [KEY, S], BF16).ap()
    if shared_rope:
        cos_sb["cosk"] = cos_sb["cosq"]
        cos_sb["sin2k"] = cos_sb["sin2q"]
    eye_sb = nc.alloc_sbuf_tensor("sb_eye", [128, 128], BF16).ap()
    perm_sb = nc.alloc_sbuf_tensor("sb_perm", [128, 128], BF16).ap()
    onescol_sb = nc.alloc_sbuf_tensor("sb_onescol", [128, 1], BF16).ap()
    onesrow_sb = nc.alloc_sbuf_tensor("sb_onesrow", [1, 128], F32R).ap()
    if mask_mode == "tril":
        dmask_sb = nc.alloc_sbuf_tensor("sb_dmask", [128, 896], BF16).ap()
    wqkv_sb = nc.alloc_sbuf_tensor("sb_wqkv", [128, FCH, MOUT * 128], BF16).ap()
    # q/k slabs: rope'd, [d, tok] per (head, block); v slabs token-major
    q_sb = {}
    k_sb = {}
    v_sb = {}
    for h in range(HPC):
        for tb in range(NTB):
            q_sb[(h, tb)] = nc.alloc_sbuf_tensor(f"sb_q_{h}_{tb}", [128, TB], BF16).ap()
            k_sb[(h, tb)] = nc.alloc_sbuf_tensor(f"sb_k_{h}_{tb}", [128, TB], BF16).ap()
            v_sb[(h, tb)] = nc.alloc_sbuf_tensor(f"sb_v_{h}_{tb}", [128, TB], BF16).ap()

    with tile.TileContext(nc) as tc:
        # tiny constants first so the first ssq matmul is never blocked
        nc.sync.dma_start(onescol_sb, onescol_d.ap())
        nc.sync.dma_start(onesrow_sb, onesrow_d.ap())
        nc.sync.dma_start(eye_sb, eye_d.ap())
        nc.sync.dma_start(perm_sb, perm_d.ap())

        from contextlib import ExitStack
        with ExitStack() as _stack:
            def _pool(name, bufs, **kw):
                return _stack.enter_context(tc.tile_pool(name=name, bufs=bufs, **kw))
            pool_x = _pool("xb", 17)
            pool_x2 = _pool("x2", 1)
            pool_small = _pool("small", 1)
            pool_nbc = _pool("nbc", 2)
            pool_rn = _pool("rn", 1)
            pool_rope = _pool("rope", 2)
            pool_pt = _pool("pt", 3)
            pool_dg = _pool("dg", 1)
            pool_oT = _pool("oT", 2)
            pool_sp = _pool("sp", 1)
            pool_gm = _pool("gm", 4)
            pool_aT = _pool("aT", 1)
            pool_we = _pool("we", 1)
            pool_res = _pool("res", 2)
            ps_sc = _pool("ps_sc", 2, space="PSUM")
            ps_pv = _pool("ps_pv", 2, space="PSUM")
            ps_sm = _pool("ps_sm", 2, space="PSUM")
            ps_misc = _pool("ps_misc", 2, space="PSUM")

            # preload the combined ln+exp activation table set once, so the
            # Ln/Exp mix never thrashes ACT_TABLE_LOADs
            from concourse.hw_specs import get_activation_tables
            _tables = list(get_activation_tables(nc.m.arch))
            _lnexp_id = _tables.index("natural_log_exp_and_others")
            nc.scalar.add_instruction(mybir.InstLoadActFuncSet(
                name="preload_lnexp", act_func_set_id=_lnexp_id, ins=[], outs=[]))

            # interleaved weight + block-0 activation loads: PE can start on
            # chunk 0 almost immediately
            first_x = []
            for c in range(FCH):
                nc.sync.dma_start(wqkv_sb[:, c, :],
                                  wqkvT.ap()[c * 128:(c + 1) * 128, :])
                xc = pool_x.tile([128, TB], BF16, tag="xb", name=f"xb0_{c}")
                nc.sync.dma_start(xc[:], xT.ap()[0, c * 128:(c + 1) * 128, 0:TB])
                first_x.append(xc)
            if mask_mode == "tril":
                nc.sync.dma_start(dmask_sb, dmask_d.ap())
            # cos/sin tables split per 512-column chunk; block tb only needs
            # chunk tb % 4, so block 0 unblocks after the first loads
            for ch in range(4):
                for name, dt_ in cos_tabs:
                    nc.sync.dma_start(cos_sb[name][:, ch * TB:(ch + 1) * TB],
                                      dt_.ap()[:, ch * TB:(ch + 1) * TB])

            wtiles = {}

            def emit_wout_loads(cs, es):
                for c in cs:
                    for e in es:
                        wt = pool_we.tile([128, 512], BF16, tag=f"we{c}_{e % 2}",
                                          name=f"we{e}_{c}")
                        nc.sync.dma_start(
                            wt[:], woutT.ap()[c * 128:(c + 1) * 128,
                                              e * 512:(e + 1) * 512])
                        wtiles[(e, c)] = wt

            # ================= Phase 1: norm + QKV + RoPE per token block ====
            def emit_block(tb):
                b = tb // 4
                s0 = (tb % 4) * TB
                if tb == 0:
                    xbs = first_x
                else:
                    xbs = []
                    for c in range(FCH):
                        xc = pool_x.tile([128, TB], BF16, tag="xb", name=f"xb{tb}_{c}")
                        nc.sync.dma_start(xc[:], xT.ap()[b, c * 128:(c + 1) * 128,
                                                         s0:s0 + TB])
                        xbs.append(xc)

                # sum-of-squares on DVE: square into 2 rotating tmps, add into
                # 2 accumulator chains, then one matmul collapses partitions
                t_ = pool_x2.tile([128, TB], BF16, tag="x2t", name=f"x2t{tb}")
                s_ = [pool_x2.tile([128, TB], BF16, tag=f"x2s{i}",
                                   name=f"x2s{tb}_{i}") for i in range(2)]
                for c in range(FCH):
                    if c < 2:
                        nc.vector.tensor_tensor(s_[c][:], xbs[c][:], xbs[c][:],
                                                AX.mult)
                    else:
                        nc.vector.tensor_tensor(t_[:], xbs[c][:], xbs[c][:],
                                                AX.mult)
                        nc.vector.tensor_tensor(s_[c % 2][:], s_[c % 2][:],
                                                t_[:], AX.add)
                nc.vector.tensor_tensor(s_[0][:], s_[0][:], s_[1][:], AX.add)

                # QKV accumulation groups; evictions trail one group behind
                accs = {}

                def emit_evict(m):
                    acc = accs.pop(m)
                    if m < 4:
                        h = m % 2
                        is_q = m < 2
                        ctab = ropen["cosq" if is_q else "cosk"]
                        stab = ropen["sin2q" if is_q else "sin2k"]
                        a_t = pool_rope.tile([128, TB], BF16, tag="ropea",
                                             name=f"ra{tb}_{m}")
                        b_t = pool_rope.tile([128, TB], BF16, tag="ropeb",
                                             name=f"rb{tb}_{m}")
                        nc.vector.tensor_tensor(a_t[:], acc[:], ctab[:], AX.mult)
                        nc.vector.tensor_tensor(b_t[:], acc[:], stab[:], AX.mult)
                        rp = ps_sc.tile([128, TB], F32, tag="sc", name=f"rp{tb}_{m}")
                        nc.tensor.matmul(rp[:], perm_sb, b_t[:], start=True, stop=True)
                        dst = q_sb[(h, tb)] if is_q else k_sb[(h, tb)]
                        nc.vector.tensor_tensor(dst[:], a_t[:], rp[:], AX.add)
                    else:
                        h = m - 4
                        vtmp = pool_rope.tile([128, TB], BF16, tag="vtmp", bufs=1,
                                              name=f"vt{tb}_{m}")
                        nc.vector.tensor_tensor(vtmp[:], acc[:], nbc[:], AX.mult)
                        for j in range(TB // 128):
                            tp = ps_sc.tile([128, 128], BF16, tag="sc",
                                            name=f"tp{tb}_{m}_{j}")
                            nc.tensor.transpose(tp[:], vtmp[:, j * 128:(j + 1) * 128],
                                                eye_sb)
                            nc.vector.tensor_copy(
                                v_sb[(h, tb)][:, j * 128:(j + 1) * 128], tp[:])

                for m in range(MOUT):
                    acc = ps_misc.tile([128, TB], F32, tag="misc", name=f"acc{tb}_{m}")
                    for c in range(FCH):
                        nc.tensor.matmul(acc[:], wqkv_sb[:, c, m * 128:(m + 1) * 128],
                                         xbs[c][:], start=(c == 0), stop=(c == FCH - 1))
                    accs[m] = acc
                    if m == 0:
                        # partition-collapse of the DVE sum-of-squares
                        ssq = ps_sm.tile([1, TB], F32, tag="sm", name=f"ssq{tb}")
                        nc.tensor.matmul(ssq[:], onescol_sb, s_[0][:],
                                         start=True, stop=True)
                        lnt = pool_small.tile([1, TB], F32, tag="ln", name=f"lnt{tb}")
                        nc.scalar.activation(lnt[:], ssq[:],
                                             mybir.ActivationFunctionType.Ln,
                                             bias=EPS, scale=1.0 / EMB)
                        invn = pool_small.tile([1, TB], F32R, tag="inv", bufs=2,
                                               name=f"invn{tb}")
                        with nc.allow_low_precision(reason="f32r bcast matmul"):
                            nc.scalar.activation(invn[:], lnt[:],
                                                 mybir.ActivationFunctionType.Exp,
                                                 scale=-0.5)
                    elif m == 1:
                        # norm broadcast + rope tables (ACT chain done during m=1)
                        nbc_ps = ps_sc.tile([128, TB], F32, tag="sc", name=f"nb{tb}")
                        nc.tensor.matmul(nbc_ps[:], onesrow_sb, invn[:],
                                         start=True, stop=True)
                        nbc = pool_nbc.tile([128, TB], BF16, tag="nbc", name=f"nbc{tb}")
                        nc.vector.tensor_copy(nbc[:], nbc_ps[:])
                        ropen = {}
                        rope_names = (("cosq", "sin2q") if shared_rope
                                      else ("cosq", "sin2q", "cosk", "sin2k"))
                        for nm in rope_names:
                            rt = pool_rn.tile([128, TB], BF16, tag=f"rn_{nm}",
                                               name=f"rn{tb}_{nm}")
                            nc.vector.tensor_tensor(rt[:], cos_sb[nm][:, s0:s0 + TB],
                                                    nbc[:], AX.mult)
                            ropen[nm] = rt
                        if shared_rope:
                            ropen["cosk"] = ropen["cosq"]
                            ropen["sin2k"] = ropen["sin2q"]
                        emit_evict(0)
                    else:
                        emit_evict(m - 1)
                emit_evict(MOUT - 1)

            for tb in range(NTB):
                emit_block(tb)
                # stream the first-use w_out tiles during phase 1 (SBUF is
                # statically reserved for them anyway; sync queue has slack).
                # e>=2 reuses the same pool tags, so those loads must wait
                # until the out-projection starts releasing tiles.
                if 1 <= tb <= 4:
                    emit_wout_loads(range((tb - 1) * 4, tb * 4), (0, 1))

            # ================= Phase 2: attention, h outer ===================
            pending = []   # deferred softmax finalize state

            def do_finalize():
                if not pending:
                    return
                b, h, qb, pacc, rec = pending.pop()
                rb_ps = ps_misc.tile([128, QF], F32, tag="misc", name=f"rp{b}{h}{qb}")
                nc.tensor.matmul(rb_ps[:], onesrow_sb, rec[:], start=True, stop=True)
                rb = pool_nbc.tile([128, QF], BF16, tag="rb", bufs=1,
                                   name=f"rb{b}{h}{qb}")
                nc.vector.tensor_copy(rb[:], rb_ps[:])
                oT = pool_oT.tile([128, QF], BF16, tag="oT", name=f"oT{b}{h}{qb}")
                nc.vector.tensor_tensor(oT[:], pacc[:], rb[:], AX.mult)
                nc.sync.dma_start(a2a_in[h].ap()[4 * b + qb, :, :], oT[:])

            def emit_attention(b, h, qb):
                tb_q = b * 4 + qb
                nkt = (4 * qb + 4) if mask_mode == "tril" else (S // KT)
                pacc = ps_pv.tile([128, QF], F32, tag="pacc", name=f"pv{b}_{h}_{qb}")
                dacc = ps_sm.tile([1, QF], F32, tag="sm", name=f"da{b}_{h}_{qb}")
                ngrp = (nkt + 3) // 4
                pts = {}
                gts = {}

                def emit_scores(kt):
                    sc = ps_sc.tile([128, QF], F32, tag="sc", name=f"sc{b}{h}{qb}_{kt}")
                    is_diag = mask_mode == "tril" and kt >= 4 * qb
                    nc.tensor.matmul(sc[:], k_sb[(h, b * 4 + kt // 4)][:, (kt % 4) * 128:(kt % 4) * 128 + 128],
                                     q_sb[(h, tb_q)][:],
                                     start=True, stop=not (mask_mode == "generic"))
                    if mask_mode == "generic":
                        gm = pool_gm.tile([128, QF], BF16, tag="gm",
                                          name=f"gm{b}{h}{qb}_{kt}")
                        nc.sync.dma_start(
                            gm[:], gmask_d.ap()[kt, :, qb * QF:(qb + 1) * QF])
                        nc.tensor.matmul(sc[:], eye_sb, gm[:], start=False, stop=True)
                    pt = pool_pt.tile([128, QF], BF16, tag="pt",
                                      name=f"pt{b}{h}{qb}_{kt}")
                    nc.scalar.activation(pt[:], sc[:],
                                         mybir.ActivationFunctionType.Exp,
                                         scale=SOFTMAX_SCALE)
                    if is_diag:
                        off = 384 - 128 * (kt - 4 * qb)
                        nc.vector.tensor_tensor(
                            pt[:], pt[:], dmask_sb[:, off:off + QF], AX.mult)
                    pts[kt] = pt
                    # denominator tree: pairs then group-of-4 sums on DVE
                    if kt % 2 == 1:
                        g = kt // 4
                        if kt % 4 == 1:
                            gt = pool_dg.tile([128, QF], BF16, tag=f"dg{g % 2}",
                                              name=f"dg{b}{h}{qb}_{g}")
                            nc.vector.tensor_tensor(gt[:], pts[kt - 1][:], pt[:],
                                                    AX.add)
                            gts[g] = gt
                        else:
                            t2 = pool_dg.tile([128, QF], BF16, tag="dh",
                                              name=f"dh{b}{h}{qb}_{g}")
                            nc.vector.tensor_tensor(t2[:], pts[kt - 1][:], pt[:],
                                                    AX.add)
                            nc.vector.tensor_tensor(gts[g][:], gts[g][:], t2[:],
                                                    AX.add)

                def emit_pv(kt):
                    pt = pts.pop(kt)
                    tb_k = b * 4 + kt // 4
                    kc = (kt % 4) * 128
                    nc.tensor.matmul(pacc[:], v_sb[(h, tb_k)][:, kc:kc + 128],
                                     pt[:], start=(kt == 0), stop=(kt == nkt - 1))
                    if kt % 4 == 3:
                        g = kt // 4
                        nc.tensor.matmul(dacc[:], onescol_sb, gts.pop(g)[:],
                                         start=(g == 0), stop=(g == ngrp - 1))

                # scores run 2 tiles ahead of the PV consumers; the previous
                # group's finalize lands after this group's first scores
                LAG = 2
                for kt in range(nkt):
                    emit_scores(kt)
                    if kt == 1:
                        do_finalize()
                    if kt >= LAG:
                        emit_pv(kt - LAG)
                for kt in range(max(0, nkt - LAG), nkt):
                    emit_pv(kt)

                # 1/denominator via exp(-ln(x)) on ACT (DVE reciprocal is slow)
                lnd = pool_small.tile([1, QF], F32, tag="ln", name=f"ld{b}{h}{qb}")
                nc.scalar.activation(lnd[:], dacc[:], mybir.ActivationFunctionType.Ln)
                rec = pool_small.tile([1, QF], F32R, tag="inv", bufs=2,
                                      name=f"rc{b}{h}{qb}")
                with nc.allow_low_precision(reason="f32r rounding for bcast matmul"):
                    nc.scalar.activation(rec[:], lnd[:],
                                         mybir.ActivationFunctionType.Exp, scale=-1.0)
                pending.append((b, h, qb, pacc, rec))

            aT = [None] * FCH

            def emit_a2a(h):
                nc.gpsimd.collective_compute(
                    "AllToAll", AX.bypass,
                    replica_groups=[list(range(NCORES))],
                    ins=[a2a_in[h][:]], outs=[a2a_out[h][:]],
                )
                # aT loads ride the gpsimd queue: they wait on the collective
                # anyway and must not block the sync queue's oT stores
                for r in range(NCORES):
                    c = 2 * r + h
                    t = pool_aT.tile([128, TPC], BF16, tag=f"aT{c}", name=f"aT{c}")
                    nc.gpsimd.dma_start(t[:], a2a_out[h].ap()[r, :, :])
                    aT[c] = t

            for h in range(HPC):
                for b in range(B):
                    for qb in range(S // QF):
                        emit_attention(b, h, qb)
                do_finalize()
                emit_a2a(h)

            # JIT tail of the w_out stream: each load unblocks as the
            # out-projection releases its tag's first tile.  Even-c tiles
            # first (consumed by the evens passes), then odd-c.
            emit_wout_loads([c for c in range(FCH) if c % 2 == 0], (2, 3))
            emit_wout_loads([c for c in range(FCH) if c % 2 == 1], (2, 3))

            # ================= Phase 3: out-projection =======================
            # all even-feature chunks (head 0 of each rank, available after
            # A2A #1) accumulate first and spill to SBUF — this fully covers
            # A2A #2's latency; the odd passes then add spill + residual.
            spills = {}
            for ep in range(EMB // 512):
                for tt in range(TPC // 128):
                    acc = ps_sc.tile([128, 512], F32, tag="sc", name=f"oe{ep}_{tt}")
                    evens = [c for c in range(FCH) if c % 2 == 0]
                    for i, c in enumerate(evens):
                        nc.tensor.matmul(acc[:], aT[c][:, tt * 128:(tt + 1) * 128],
                                         wtiles[(ep, c)][:], start=(i == 0),
                                         stop=(i == len(evens) - 1))
                    sp = pool_sp.tile([128, 512], BF16, tag=f"sp{ep}_{tt}",
                                      name=f"sp{ep}_{tt}")
                    nc.vector.tensor_copy(sp[:], acc[:])
                    spills[(ep, tt)] = sp
            for ep in range(EMB // 512):
                for tt in range(TPC // 128):
                    acc = ps_pv.tile([128, 512], F32, tag="pacc", name=f"oo{ep}_{tt}")
                    odds = [c for c in range(FCH) if c % 2 == 1]
                    for i, c in enumerate(odds):
                        nc.tensor.matmul(acc[:], aT[c][:, tt * 128:(tt + 1) * 128],
                                         wtiles[(ep, c)][:], start=(i == 0),
                                         stop=(i == len(odds) - 1))
                    # residual loads ride the (idle) ACT queue so the sync
                    # queue's JIT w_out tail can't delay them
                    res_t = pool_res.tile([128, 512], BF16, tag="res",
                                          name=f"rs{ep}_{tt}")
                    nc.scalar.dma_start(
                        res_t[:], xres.ap()[tt * 128:(tt + 1) * 128,
                                            ep * 512:(ep + 1) * 512])
                    o = pool_res.tile([128, 512], F32, tag="o", name=f"o{ep}_{tt}")
                    nc.vector.tensor_tensor(o[:], acc[:],
                                            spills.pop((ep, tt))[:], AX.add)
                    nc.vector.tensor_tensor(o[:], o[:], res_t[:], AX.add)
                    nc.sync.dma_start(
                        out_ext.ap()[tt * 128:(tt + 1) * 128,
                                     ep * 512:(ep + 1) * 512], o[:])

    nc.compile()
    return nc


_GRAPH_CACHE = {}


def _get_graph(mask_mode, shared_rope):
    key = (mask_mode, shared_rope)
    if key not in _GRAPH_CACHE:
        _GRAPH_CACHE[key] = build_graph(mask_mode, shared_rope)
    return _GRAPH_CACHE[key]


def kernel(**inputs):
    emb = np.asarray(inputs["embeddings"], dtype=np.float32)       # [B, S, EMB]
    cos = np.asarray(inputs["cos_buffer"], dtype=np.float32)       # [2,1,1,S,KEY]
    sin = np.asarray(inputs["sin_buffer"], dtype=np.float32)
    causal = np.asarray(inputs["causal_buffer"])[0, 0]             # [S, S] bool
    wqkv = np.asarray(inputs["w_qkv"], dtype=np.float32)           # [6144, EMB]
    wout = np.asarray(inputs["w_out"], dtype=np.float32)           # [EMB, EMB]

    tril = np.tril(np.ones((S, S), dtype=bool))
    if np.array_equal(causal, tril):
        mask_mode = "tril"
    elif causal.all():
        mask_mode = "none"
    else:
        mask_mode = "generic"

    shared_rope = bool(np.array_equal(cos[0], cos[1])
                       and np.array_equal(sin[0], sin[1]))
    nc = _get_graph(mask_mode, shared_rope)

    bf = ml_dtypes.bfloat16
    xT = np.ascontiguousarray(emb.transpose(0, 2, 1)).astype(bf)   # [B, EMB, S]
    x2d = emb.reshape(T, EMB)
    wq = wqkv[0:EMB].reshape(NH, KEY, EMB)
    wk = wqkv[EMB:2 * EMB].reshape(NH, KEY, EMB)
    wv = wqkv[2 * EMB:].reshape(NH, HEAD, EMB)
    woutT_a = np.ascontiguousarray(wout.T).astype(bf)
    cosq_a = np.ascontiguousarray(cos[0, 0, 0].T).astype(bf)
    sin2q_a = np.ascontiguousarray(np.roll(sin[0, 0, 0].T, -64, axis=0)).astype(bf)
    cosk_a = np.ascontiguousarray(cos[1, 0, 0].T).astype(bf)
    sin2k_a = np.ascontiguousarray(np.roll(sin[1, 0, 0].T, -64, axis=0)).astype(bf)
    eye_a = np.eye(128, dtype=np.float32).astype(bf)
    perm_a = np.roll(np.eye(128, dtype=np.float32), 64, axis=0).astype(bf)
    onescol_a = np.ones((128, 1), np.float32).astype(bf)
    onesrow_a = np.ones((1, 128), np.float32)

    if mask_mode == "tril":
        p = np.arange(128)[:, None]
        u = np.arange(896)[None, :]
        dmask_a = np.where(u >= p + 384, 1.0, 0.0).astype(bf)      # [128, 896]
    elif mask_mode == "generic":
        # additive mask in [kt, p, q] layout: keep where causal[q, k]
        cz = causal.T.reshape(S // KT, KT, S)                      # [kt, p(k), q]
        gmask_a = np.where(cz, 0.0, NEG_BIG).astype(bf)

    in_maps = []
    for c in range(NCORES):
        h0, h1 = 2 * c, 2 * c + 1
        wshard = np.concatenate(
            [wq[h0], wq[h1], wk[h0], wk[h1], wv[h0], wv[h1]], axis=0)   # [768, EMB]
        m = {
            "xT": xT,
            "xres": np.ascontiguousarray(x2d[c * TPC:(c + 1) * TPC]).astype(bf),
            "wqkvT": np.ascontiguousarray(wshard.T).astype(bf),
            "woutT": woutT_a,
            "cosq": cosq_a, "sin2q": sin2q_a,
            "eye": eye_a, "perm": perm_a,
            "onescol": onescol_a, "onesrow": onesrow_a,
        }
        if not shared_rope:
            m["cosk"] = cosk_a
            m["sin2k"] = sin2k_a
        if mask_mode == "tril":
            m["dmask"] = dmask_a
        elif mask_mode == "generic":
            m["gmask"] = gmask_a
        in_maps.append(m)

    trace = os.environ.get("BASS_KERNEL_PROFILE") == "1"
    res = run_bass_kernel_spmd(nc, in_maps, core_ids=list(range(NCORES)),
                               trace=trace)
    if trace:
        kernel.last_exec_time_ns = res.exec_time_ns
        kernel.last_results = res

    outs = [np.asarray(res.results[c]["out"], dtype=np.float32)
            for c in range(NCORES)]
    full = np.concatenate(outs, axis=0).reshape(B, S, EMB)
    return full


# revision 48
# speedup vs baseline: 1.0649x; 1.0532x over previous
"""Distributed Trainium2 Bass kernel for a dense transformer self-attention block.

Reference computation (fp32):
  normed = x * rsqrt(mean(x^2, -1) + 1e-5)
  qkv = normed @ w_qkv.T ; split into q,k (RoPE'd) and v, 16 heads x 128
  attn = softmax(causal(q k^T / sqrt(128)))
  out  = x + (attn @ v merged) @ w_out.T

Sharding across 8 NeuronCores: tensor-parallel by head (2 heads/core) for
QKV projection + attention; AllToAll converts the head-sharded attention
output to a sequence-sharded layout; the out-projection + residual run on
each core's 512-token slice.  Host-side gather is pure concatenation.

v2 notes (PE is the bottleneck: keep matmul count + columns minimal):
  - sum-of-squares via DVE accumulation + 1 matmul/block (was 16)
  - RoPE: one perm-matmul + DVE add (was perm+eye matmuls)
  - causal diag mask: multiplicative 0/1 on DVE post-exp (was PE matmul)
  - softmax denominator: DVE group-of-4 tree + 1 matmul/group (was 1/kt)
  - deferred per-group softmax finalize so PE never waits the ACT chain
  - w_out loads stream during phase 1; aT loads ride the gpsimd queue so
    the sync queue never head-of-line blocks oT stores
  - out-projection: all even-feature accumulations first (covers A2A#2)
"""

import os
import sys

if "/opt/trn_rl_repo" not in sys.path:
    sys.path.insert(0, "/opt/trn_rl_repo")

import numpy as np
import ml_dtypes

import concourse.bass as bass
import concourse.mybir as mybir
from concourse import bacc, tile
from concourse.bass_utils import run_bass_kernel_spmd

EMB, KEY, HEAD, NH = 2048, 128, 128, 16
B, S = 2, 2048
NCORES = 8
HPC = NH // NCORES          # heads per core = 2
T = B * S                   # 4096 tokens
TPC = T // NCORES           # 512 tokens per core after A2A
TB = 512                    # token block for QKV phase
NTB = T // TB               # 8 blocks
QF = 512                    # query free-dim in attention
KT = 128                    # key tile (partition dim)
FCH = EMB // 128            # 16 feature chunks
MOUT = 6                    # qkv output tiles per core (q0 q1 k0 k1 v0 v1)
EPS = 1e-5
SOFTMAX_SCALE = 1.0 / float(np.sqrt(KEY))
NEG_BIG = -30000.0

BF16 = mybir.dt.bfloat16
F32 = mybir.dt.float32
F32R = mybir.dt.float32r
AX = mybir.AluOpType


def build_graph(mask_mode: str, shared_rope: bool = True):
    """mask_mode: 'tril' (causal fast path), 'none' (no masking),
    'generic' (arbitrary additive mask input).  shared_rope: the q and k
    cos/sin tables are identical (true for the reference setup), so keep
    only one SBUF copy of each."""
    nc = bacc.Bacc("TRN2", target_bir_lowering=False, debug=False,
                   num_devices=NCORES)

    # register the extra activation-bias constant (Ln bias = EPS)
    _ct = nc.alloc_sbuf_tensor(f"const-extra-eps", [128, 1], F32)
    nc.gpsimd.memset(_ct.ap(), EPS)
    nc.const_aps.aps[(F32, EPS)] = _ct.ap()
    nc.all_engine_barrier()

    xT = nc.dram_tensor("xT", [B, EMB, S], BF16, kind="ExternalInput")
    xres = nc.dram_tensor("xres", [TPC, EMB], BF16, kind="ExternalInput")
    wqkvT = nc.dram_tensor("wqkvT", [EMB, MOUT * 128], BF16, kind="ExternalInput")
    woutT = nc.dram_tensor("woutT", [EMB, EMB], BF16, kind="ExternalInput")
    cos_tabs = [("cosq", nc.dram_tensor("cosq", [KEY, S], BF16,
                                        kind="ExternalInput")),
                ("sin2q", nc.dram_tensor("sin2q", [KEY, S], BF16,
                                         kind="ExternalInput"))]
    if not shared_rope:
        cos_tabs += [("cosk", nc.dram_tensor("cosk", [KEY, S], BF16,
                                             kind="ExternalInput")),
                     ("sin2k", nc.dram_tensor("sin2k", [KEY, S], BF16,
                                              kind="ExternalInput"))]
    eye_d = nc.dram_tensor("eye", [128, 128], BF16, kind="ExternalInput")
    perm_d = nc.dram_tensor("perm", [128, 128], BF16, kind="ExternalInput")
    onescol_d = nc.dram_tensor("onescol", [128, 1], BF16, kind="ExternalInput")
    onesrow_d = nc.dram_tensor("onesrow", [1, 128], F32R, kind="ExternalInput")
    if mask_mode == "tril":
        # single shifted mask base: window [384-128t : 896-128t] is the 0/1
        # multiplicative mask for diagonal tile t
        dmask_d = nc.dram_tensor("dmask", [128, 896], BF16, kind="ExternalInput")
    elif mask_mode == "generic":
        gmask_d = nc.dram_tensor("gmask", [S // KT, KT, S], BF16, kind="ExternalInput")
    out_ext = nc.dram_tensor("out", [TPC, EMB], BF16, kind="ExternalOutput")

    # A2A split by head so the first collective overlaps the second head's
    # attention and the out-projection can start on half the features.
    a2a_in = [nc.dram_tensor(f"a2a_in{h}", [NCORES, HEAD, TPC], BF16)
              for h in range(HPC)]
    a2a_out = [nc.dram_tensor(f"a2a_out{h}", [NCORES, HEAD, TPC], BF16)
               for h in range(HPC)]

    # ---- static SBUF residents ----
    cos_sb = {}
    for name, _ in cos_tabs:
        cos_sb[name] = nc.alloc_sbuf_tensor(f"sb_{name}", [KEY, S], BF16).ap()
    if shared_rope:
        cos_sb["cosk"] = cos_sb["cosq"]
        cos_sb["sin2k"] = cos_sb["sin2q"]
    eye_sb = nc.alloc_sbuf_tensor("sb_eye", [128, 128], BF16).ap()
    perm_sb = nc.alloc_sbuf_tensor("sb_perm", [128, 128], BF16).ap()
    onescol_sb = nc.alloc_sbuf_tensor("sb_onescol", [128, 1], BF16).ap()
    onesrow_sb = nc.alloc_sbuf_tensor("sb_onesrow", [1, 128], F32R).ap()
    if mask_mode == "tril":
        dmask_sb = nc.alloc_sbuf_tensor("sb_dmask", [128, 896], BF16).ap()
    wqkv_sb = nc.alloc_sbuf_tensor("sb_wqkv", [128, FCH, MOUT * 128], BF16).ap()
    # q/k slabs: rope'd, [d, tok] per (head, block); v slabs token-major
    q_sb = {}
    k_sb = {}
    v_sb = {}
    for h in range(HPC):
        for tb in range(NTB):
            q_sb[(h, tb)] = nc.alloc_sbuf_tensor(f"sb_q_{h}_{tb}", [128, TB], BF16).ap()
            k_sb[(h, tb)] = nc.alloc_sbuf_tensor(f"sb_k_{h}_{tb}", [128, TB], BF16).ap()
            v_sb[(h, tb)] = nc.alloc_sbuf_tensor(f"sb_v_{h}_{tb}", [128, TB], BF16).ap()

    with tile.TileContext(nc) as tc:
        # tiny constants first so the first ssq matmul is never blocked
        nc.sync.dma_start(onescol_sb, onescol_d.ap())
        nc.sync.dma_start(onesrow_sb, onesrow_d.ap())
        nc.sync.dma_start(eye_sb, eye_d.ap())
        nc.sync.dma_start(perm_sb, perm_d.ap())

        from contextlib import ExitStack
        with ExitStack() as _stack:
            def _pool(name, bufs, **kw):
                return _stack.enter_context(tc.tile_pool(name=name, bufs=bufs, **kw))
            pool_x = _pool("xb", 20)
            pool_x2 = _pool("x2", 1)
            pool_small = _pool("small", 1)
            pool_nbc = _pool("nbc", 2)
            pool_rn = _pool("rn", 1)
            pool_rope = _pool("rope", 2)
            pool_pt = _pool("pt", 3)
            pool_dg = _pool("dg", 1)
            pool_oT = _pool("oT", 2)
            pool_sp = _pool("sp", 1)
            pool_gm = _pool("gm", 4)
            pool_aT = _pool("aT", 1)
            pool_we = _pool("we", 1)
            pool_res = _pool("res", 2)
            # 8 PSUM banks: accs/scores/evens ring (4) + pacc/odds (2) +
            # ssq/transients/dacc (2)
            ps_pv = _pool("ps_pv", 2, space="PSUM")
            ps_sm = _pool("ps_sm", 2, space="PSUM")
            ps_misc = _pool("ps_misc", 4, space="PSUM")
            ps_sc = ps_misc

            # preload the combined ln+exp activation table set once, so the
            # Ln/Exp mix never thrashes ACT_TABLE_LOADs
            from concourse.hw_specs import get_activation_tables
            _tables = list(get_activation_tables(nc.m.arch))
            _lnexp_id = _tables.index("natural_log_exp_and_others")
            nc.scalar.add_instruction(mybir.InstLoadActFuncSet(
                name="preload_lnexp", act_func_set_id=_lnexp_id, ins=[], outs=[]))

            # interleaved weight + block-0 activation loads: PE can start on
            # chunk 0 almost immediately
            first_x = []
            for c in range(FCH):
                xc = pool_x.tile([128, TB], BF16, tag="xb", name=f"xb0_{c}")
                nc.sync.dma_start(xc[:], xT.ap()[0, c * 128:(c + 1) * 128, 0:TB])
                first_x.append(xc)
                nc.sync.dma_start(wqkv_sb[:, c, :],
                                  wqkvT.ap()[c * 128:(c + 1) * 128, :])
            if mask_mode == "tril":
                nc.sync.dma_start(dmask_sb, dmask_d.ap())
            # cos/sin tables split per 512-column chunk; block tb only needs
            # chunk tb % 4, so block 0 unblocks after the first loads
            for ch in range(4):
                for name, dt_ in cos_tabs:
                    nc.sync.dma_start(cos_sb[name][:, ch * TB:(ch + 1) * TB],
                                      dt_.ap()[:, ch * TB:(ch + 1) * TB])

            wtiles = {}

            def emit_wout_loads(cs, es, eng=None):
                for c in cs:
                    for e in es:
                        wt = pool_we.tile([128, 512], BF16, tag=f"we{c}_{e % 2}",
                                          name=f"we{e}_{c}")
                        (eng or nc.sync).dma_start(
                            wt[:], woutT.ap()[c * 128:(c + 1) * 128,
                                              e * 512:(e + 1) * 512])
                        wtiles[(e, c)] = wt

            # ================= Phase 1: norm + QKV + RoPE per token block ====
            def emit_block(tb):
                b = tb // 4
                s0 = (tb % 4) * TB
                if tb == 0:
                    xbs = first_x
                else:
                    xbs = []
                    for c in range(FCH):
                        xc = pool_x.tile([128, TB], BF16, tag="xb", name=f"xb{tb}_{c}")
                        nc.sync.dma_start(xc[:], xT.ap()[b, c * 128:(c + 1) * 128,
                                                         s0:s0 + TB])
                        xbs.append(xc)

                # sum-of-squares on DVE: square into 2 rotating tmps, add into
                # 2 accumulator chains, then one matmul collapses partitions
                t_ = pool_x2.tile([128, TB], BF16, tag="x2t", name=f"x2t{tb}")
                s_ = [pool_x2.tile([128, TB], BF16, tag=f"x2s{i}",
                                   name=f"x2s{tb}_{i}") for i in range(2)]
                for c in range(FCH):
                    if c < 2:
                        nc.vector.tensor_tensor(s_[c][:], xbs[c][:], xbs[c][:],
                                                AX.mult)
                    else:
                        nc.vector.tensor_tensor(t_[:], xbs[c][:], xbs[c][:],
                                                AX.mult)
                        nc.vector.tensor_tensor(s_[c % 2][:], s_[c % 2][:],
                                                t_[:], AX.add)
                nc.vector.tensor_tensor(s_[0][:], s_[0][:], s_[1][:], AX.add)

                # QKV accumulation groups; evictions trail one group behind
                accs = {}

                def emit_evict(m):
                    acc = accs.pop(m)
                    if m < 4:
                        h = m % 2
                        is_q = m < 2
                        ctab = ropen["cosq" if is_q else "cosk"]
                        stab = ropen["sin2q" if is_q else "sin2k"]
                        a_t = pool_rope.tile([128, TB], BF16, tag="ropea",
                                             name=f"ra{tb}_{m}")
                        b_t = pool_rope.tile([128, TB], BF16, tag="ropeb",
                                             name=f"rb{tb}_{m}")
                        nc.vector.tensor_tensor(a_t[:], acc[:], ctab[:], AX.mult)
                        nc.vector.tensor_tensor(b_t[:], acc[:], stab[:], AX.mult)
                        rp = ps_pv.tile([128, TB], F32, tag="pacc", name=f"rp{tb}_{m}")
                        nc.tensor.matmul(rp[:], perm_sb, b_t[:], start=True, stop=True)
                        dst = q_sb[(h, tb)] if is_q else k_sb[(h, tb)]
                        nc.vector.tensor_tensor(dst[:], a_t[:], rp[:], AX.add)
                    else:
                        h = m - 4
                        vtmp = pool_rope.tile([128, TB], BF16, tag="vtmp", bufs=1,
                                              name=f"vt{tb}_{m}")
                        nc.vector.tensor_tensor(vtmp[:], acc[:], nbc[:], AX.mult)
                        # [128, 2*TB] bf16 has the same byte footprint as the
                        # misc ring's [128, TB] f32 slot, so it shares slots
                        tp = ps_misc.tile([128, 2 * TB], BF16, tag="misc",
                                          name=f"tp{tb}_{m}")
                        for j in range(TB // 128):
                            nc.tensor.transpose(tp[:, j * 128:(j + 1) * 128],
                                                vtmp[:, j * 128:(j + 1) * 128],
                                                eye_sb)
                        nc.vector.tensor_copy(v_sb[(h, tb)][:], tp[:, 0:TB])

                # norm chain is deliberately LATE (ssq consumes the serial DVE
                # square/add chain, which ends ~12us into the block): ssq at
                # m=2's end, broadcast + rope tables + first eviction at m=3's
                # end.  The 4-deep acc ring makes the longer eviction lag legal.
                for m in range(MOUT):
                    acc = ps_misc.tile([128, TB], F32, tag="misc", name=f"acc{tb}_{m}")
                    for c in range(FCH):
                        nc.tensor.matmul(acc[:], wqkv_sb[:, c, m * 128:(m + 1) * 128],
                                         xbs[c][:], start=(c == 0), stop=(c == FCH - 1))
                    accs[m] = acc
                    if m == 2:
                        # partition-collapse of the DVE sum-of-squares
                        ssq = ps_sm.tile([1, TB], F32, tag="sm", name=f"ssq{tb}")
                        nc.tensor.matmul(ssq[:], onescol_sb, s_[0][:],
                                         start=True, stop=True)
                        lnt = pool_small.tile([1, TB], F32, tag="ln", name=f"lnt{tb}")
                        nc.scalar.activation(lnt[:], ssq[:],
                                             mybir.ActivationFunctionType.Ln,
                                             bias=EPS, scale=1.0 / EMB)
                        invn = pool_small.tile([1, TB], F32R, tag="inv", bufs=2,
                                               name=f"invn{tb}")
                        with nc.allow_low_precision(reason="f32r bcast matmul"):
                            nc.scalar.activation(invn[:], lnt[:],
                                                 mybir.ActivationFunctionType.Exp,
                                                 scale=-0.5)
                    elif m == 3:
                        nbc_ps = ps_pv.tile([128, TB], F32, tag="pacc", name=f"nb{tb}")
                        nc.tensor.matmul(nbc_ps[:], onesrow_sb, invn[:],
                                         start=True, stop=True)
                        nbc = pool_nbc.tile([128, TB], BF16, tag="nbc", name=f"nbc{tb}")
                        nc.vector.tensor_copy(nbc[:], nbc_ps[:])
                        ropen = {}
                        rope_names = (("cosq", "sin2q") if shared_rope
                                      else ("cosq", "sin2q", "cosk", "sin2k"))
                        for nm in rope_names:
                            rt = pool_rn.tile([128, TB], BF16, tag=f"rn_{nm}",
                                               name=f"rn{tb}_{nm}")
                            nc.vector.tensor_tensor(rt[:], cos_sb[nm][:, s0:s0 + TB],
                                                    nbc[:], AX.mult)
                            ropen[nm] = rt
                        if shared_rope:
                            ropen["cosk"] = ropen["cosq"]
                            ropen["sin2k"] = ropen["sin2q"]
                        emit_evict(0)
                    elif m == 4:
                        emit_evict(1)
                    elif m == 5:
                        emit_evict(2)
                        emit_evict(3)
                emit_evict(4)
                emit_evict(5)

            for tb in range(NTB):
                emit_block(tb)
                # stream the first-use w_out tiles during phase 1 (SBUF is
                # statically reserved for them anyway; sync queue has slack).
                # e>=2 reuses the same pool tags, so those loads must wait
                # until the out-projection starts releasing tiles.
                if 1 <= tb <= 4:
                    emit_wout_loads(range((tb - 1) * 4, tb * 4), (0, 1))

            # ================= Phase 2: attention, h outer ===================
            pending = []   # deferred softmax finalize state

            def do_finalize():
                if not pending:
                    return
                b, h, qb, pacc, rec = pending.pop()
                rb_ps = ps_misc.tile([128, QF], F32, tag="misc", name=f"rp{b}{h}{qb}")
                nc.tensor.matmul(rb_ps[:], onesrow_sb, rec[:], start=True, stop=True)
                rb = pool_nbc.tile([128, QF], BF16, tag="rb", bufs=1,
                                   name=f"rb{b}{h}{qb}")
                nc.vector.tensor_copy(rb[:], rb_ps[:])
                oT = pool_oT.tile([128, QF], BF16, tag="oT", name=f"oT{b}{h}{qb}")
                nc.vector.tensor_tensor(oT[:], pacc[:], rb[:], AX.mult)
                nc.sync.dma_start(a2a_in[h].ap()[4 * b + qb, :, :], oT[:])

            def emit_attention(b, h, qb):
                tb_q = b * 4 + qb
                nkt = (4 * qb + 4) if mask_mode == "tril" else (S // KT)
                pacc = ps_pv.tile([128, QF], F32, tag="pacc", name=f"pv{b}_{h}_{qb}")
                dacc = ps_sm.tile([1, QF], F32, tag="sm", name=f"da{b}_{h}_{qb}")
                ngrp = (nkt + 3) // 4
                pts = {}
                gts = {}

                def emit_scores(kt):
                    sc = ps_sc.tile([128, QF], F32, tag="misc", name=f"sc{b}{h}{qb}_{kt}")
                    is_diag = mask_mode == "tril" and kt >= 4 * qb
                    nc.tensor.matmul(sc[:], k_sb[(h, b * 4 + kt // 4)][:, (kt % 4) * 128:(kt % 4) * 128 + 128],
                                     q_sb[(h, tb_q)][:],
                                     start=True, stop=not (mask_mode == "generic"))
                    if mask_mode == "generic":
                        gm = pool_gm.tile([128, QF], BF16, tag="gm",
                                          name=f"gm{b}{h}{qb}_{kt}")
                        nc.sync.dma_start(
                            gm[:], gmask_d.ap()[kt, :, qb * QF:(qb + 1) * QF])
                        nc.tensor.matmul(sc[:], eye_sb, gm[:], start=False, stop=True)
                    pt = pool_pt.tile([128, QF], BF16, tag="pt",
                                      name=f"pt{b}{h}{qb}_{kt}")
                    nc.scalar.activation(pt[:], sc[:],
                                         mybir.ActivationFunctionType.Exp,
                                         scale=SOFTMAX_SCALE)
                    if is_diag:
                        off = 384 - 128 * (kt - 4 * qb)
                        nc.vector.tensor_tensor(
                            pt[:], pt[:], dmask_sb[:, off:off + QF], AX.mult)
                    pts[kt] = pt
                    # denominator tree: pairs then group-of-4 sums on DVE
                    if kt % 2 == 1:
                        g = kt // 4
                        if kt % 4 == 1:
                            gt = pool_dg.tile([128, QF], BF16, tag=f"dg{g % 2}",
                                              name=f"dg{b}{h}{qb}_{g}")
                            nc.vector.tensor_tensor(gt[:], pts[kt - 1][:], pt[:],
                                                    AX.add)
                            gts[g] = gt
                        else:
                            t2 = pool_dg.tile([128, QF], BF16, tag="dh",
                                              name=f"dh{b}{h}{qb}_{g}")
                            nc.vector.tensor_tensor(t2[:], pts[kt - 1][:], pt[:],
                                                    AX.add)
                            nc.vector.tensor_tensor(gts[g][:], gts[g][:], t2[:],
                                                    AX.add)

                def emit_pv(kt):
                    pt = pts.pop(kt)
                    tb_k = b * 4 + kt // 4
                    kc = (kt % 4) * 128
                    nc.tensor.matmul(pacc[:], v_sb[(h, tb_k)][:, kc:kc + 128],
                                     pt[:], start=(kt == 0), stop=(kt == nkt - 1))
                    if kt % 4 == 3:
                        g = kt // 4
                        nc.tensor.matmul(dacc[:], onescol_sb, gts.pop(g)[:],
                                         start=(g == 0), stop=(g == ngrp - 1))

                # scores run 2 tiles ahead of the PV consumers; the previous
                # group's finalize lands after this group's first scores
                LAG = 2
                for kt in range(nkt):
                    emit_scores(kt)
                    if kt == 1:
                        do_finalize()
                    if kt >= LAG:
                        emit_pv(kt - LAG)
                for kt in range(max(0, nkt - LAG), nkt):
                    emit_pv(kt)

                # 1/denominator via exp(-ln(x)) on ACT (DVE reciprocal is slow)
                lnd = pool_small.tile([1, QF], F32, tag="ln", name=f"ld{b}{h}{qb}")
                nc.scalar.activation(lnd[:], dacc[:], mybir.ActivationFunctionType.Ln)
                rec = pool_small.tile([1, QF], F32R, tag="inv", bufs=2,
                                      name=f"rc{b}{h}{qb}")
                with nc.allow_low_precision(reason="f32r rounding for bcast matmul"):
                    nc.scalar.activation(rec[:], lnd[:],
                                         mybir.ActivationFunctionType.Exp, scale=-1.0)
                pending.append((b, h, qb, pacc, rec))

            aT = [None] * FCH

            def emit_a2a(h):
                nc.gpsimd.collective_compute(
                    "AllToAll", AX.bypass,
                    replica_groups=[list(range(NCORES))],
                    ins=[a2a_in[h][:]], outs=[a2a_out[h][:]],
                )
                # aT loads ride the gpsimd queue: they wait on the collective
                # anyway and must not block the sync queue's oT stores
                for r in range(NCORES):
                    c = 2 * r + h
                    t = pool_aT.tile([128, TPC], BF16, tag=f"aT{c}", name=f"aT{c}")
                    nc.gpsimd.dma_start(t[:], a2a_out[h].ap()[r, :, :])
                    aT[c] = t

            for h in range(HPC):
                for b in range(B):
                    for qb in range(S // QF):
                        emit_attention(b, h, qb)
                do_finalize()
                emit_a2a(h)

            # JIT tail of the w_out stream: each load unblocks as the
            # out-projection releases its tag's first tile.  Even-c tiles
            # first (consumed by the evens passes), then odd-c.  These ride
            # the gpsimd queue so they cannot delay anything on sync.
            emit_wout_loads([c for c in range(FCH) if c % 2 == 0], (2, 3),
                            eng=nc.gpsimd)
            emit_wout_loads([c for c in range(FCH) if c % 2 == 1], (2, 3),
                            eng=nc.gpsimd)

            # ================= Phase 3: out-projection =======================
            # all even-feature chunks (head 0 of each rank, available after
            # A2A #1) accumulate first and spill to SBUF — this fully covers
            # A2A #2's latency; the odd passes then add spill + residual.
            spills = {}
            for ep in range(EMB // 512):
                for tt in range(TPC // 128):
                    acc = ps_sc.tile([128, 512], F32, tag="misc", name=f"oe{ep}_{tt}")
                    evens = [c for c in range(FCH) if c % 2 == 0]
                    for i, c in enumerate(evens):
                        nc.tensor.matmul(acc[:], aT[c][:, tt * 128:(tt + 1) * 128],
                                         wtiles[(ep, c)][:], start=(i == 0),
                                         stop=(i == len(evens) - 1))
                    sp = pool_sp.tile([128, 512], BF16, tag=f"sp{ep}_{tt}",
                                      name=f"sp{ep}_{tt}")
                    nc.vector.tensor_copy(sp[:], acc[:])
                    spills[(ep, tt)] = sp
            for ep in range(EMB // 512):
                for tt in range(TPC // 128):
                    acc = ps_pv.tile([128, 512], F32, tag="pacc", name=f"oo{ep}_{tt}")
                    odds = [c for c in range(FCH) if c % 2 == 1]
                    for i, c in enumerate(odds):
                        nc.tensor.matmul(acc[:], aT[c][:, tt * 128:(tt + 1) * 128],
                                         wtiles[(ep, c)][:], start=(i == 0),
                                         stop=(i == len(odds) - 1))
                    # residual loads ride the (idle) ACT queue so the sync
                    # queue's JIT w_out tail can't delay them
                    res_t = pool_res.tile([128, 512], BF16, tag="res",
                                          name=f"rs{ep}_{tt}")
                    nc.scalar.dma_start(
                        res_t[:], xres.ap()[tt * 128:(tt + 1) * 128,
                                            ep * 512:(ep + 1) * 512])
                    o = pool_res.tile([128, 512], BF16, tag="o", name=f"o{ep}_{tt}")
                    nc.vector.tensor_tensor(o[:], acc[:],
                                            spills.pop((ep, tt))[:], AX.add)
                    nc.vector.tensor_tensor(o[:], o[:], res_t[:], AX.add)
                    nc.sync.dma_start(
                        out_ext.ap()[tt * 128:(tt + 1) * 128,
                                     ep * 512:(ep + 1) * 512], o[:])

    nc.compile()
    return nc


_GRAPH_CACHE = {}


def _get_graph(mask_mode, shared_rope):
    key = (mask_mode, shared_rope)
    if key not in _GRAPH_CACHE:
        _GRAPH_CACHE[key] = build_graph(mask_mode, shared_rope)
    return _GRAPH_CACHE[key]


def kernel(**inputs):
    emb = np.asarray(inputs["embeddings"], dtype=np.float32)       # [B, S, EMB]
    cos = np.asarray(inputs["cos_buffer"], dtype=np.float32)       # [2,1,1,S,KEY]
    sin = np.asarray(inputs["sin_buffer"], dtype=np.float32)
    causal = np.asarray(inputs["causal_buffer"])[0, 0]             # [S, S] bool
    wqkv = np.asarray(inputs["w_qkv"], dtype=np.float32)           # [6144, EMB]
    wout = np.asarray(inputs["w_out"], dtype=np.float32)           # [EMB, EMB]

    tril = np.tril(np.ones((S, S), dtype=bool))
    if np.array_equal(causal, tril):
        mask_mode = "tril"
    elif causal.all():
        mask_mode = "none"
    else:
        mask_mode = "generic"

    shared_rope = bool(np.array_equal(cos[0], cos[1])
                       and np.array_equal(sin[0], sin[1]))
    nc = _get_graph(mask_mode, shared_rope)

    bf = ml_dtypes.bfloat16
    xT = np.ascontiguousarray(emb.transpose(0, 2, 1)).astype(bf)   # [B, EMB, S]
    x2d = emb.reshape(T, EMB)
    wq = wqkv[0:EMB].reshape(NH, KEY, EMB)
    wk = wqkv[EMB:2 * EMB].reshape(NH, KEY, EMB)
    wv = wqkv[2 * EMB:].reshape(NH, HEAD, EMB)
    woutT_a = np.ascontiguousarray(wout.T).astype(bf)
    cosq_a = np.ascontiguousarray(cos[0, 0, 0].T).astype(bf)
    sin2q_a = np.ascontiguousarray(np.roll(sin[0, 0, 0].T, -64, axis=0)).astype(bf)
    cosk_a = np.ascontiguousarray(cos[1, 0, 0].T).astype(bf)
    sin2k_a = np.ascontiguousarray(np.roll(sin[1, 0, 0].T, -64, axis=0)).astype(bf)
    eye_a = np.eye(128, dtype=np.float32).astype(bf)
    perm_a = np.roll(np.eye(128, dtype=np.float32), 64, axis=0).astype(bf)
    onescol_a = np.ones((128, 1), np.float32).astype(bf)
    onesrow_a = np.ones((1, 128), np.float32)

    if mask_mode == "tril":
        p = np.arange(128)[:, None]
        u = np.arange(896)[None, :]
        dmask_a = np.where(u >= p + 384, 1.0, 0.0).astype(bf)      # [128, 896]
    elif mask_mode == "generic":
        # additive mask in [kt, p, q] layout: keep where causal[q, k]
        cz = causal.T.reshape(S // KT, KT, S)                      # [kt, p(k), q]
        gmask_a = np.where(cz, 0.0, NEG_BIG).astype(bf)

    in_maps = []
    for c in range(NCORES):
        h0, h1 = 2 * c, 2 * c + 1
        wshard = np.concatenate(
            [wq[h0], wq[h1], wk[h0], wk[h1], wv[h0], wv[h1]], axis=0)   # [768, EMB]
        m = {
            "xT": xT,
            "xres": np.ascontiguousarray(x2d[c * TPC:(c + 1) * TPC]).astype(bf),
            "wqkvT": np.ascontiguousarray(wshard.T).astype(bf),
            "woutT": woutT_a,
            "cosq": cosq_a, "sin2q": sin2q_a,
            "eye": eye_a, "perm": perm_a,
            "onescol": onescol_a, "onesrow": onesrow_a,
        }
        if not shared_rope:
            m["cosk"] = cosk_a
            m["sin2k"] = sin2k_a
        if mask_mode == "tril":
            m["dmask"] = dmask_a
        elif mask_mode == "generic":
            m["gmask"] = gmask_a
        in_maps.append(m)

    trace = os.environ.get("BASS_KERNEL_PROFILE") == "1"
    res = run_bass_kernel_spmd(nc, in_maps, core_ids=list(range(NCORES)),
                               trace=trace)
    if trace:
        kernel.last_exec_time_ns = res.exec_time_ns
        kernel.last_results = res

    outs = [np.asarray(res.results[c]["out"], dtype=np.float32)
            for c in range(NCORES)]
    full = np.concatenate(outs, axis=0).reshape(B, S, EMB)
    return full
